# revision 51
# baseline (speedup 1.0000x reference)
"""Trainium2 Bass kernel for BaichuanAttention (hidden=5120, 40 heads, b=2, s=2048).

Tensor-parallel over heads across 8 NeuronCores, bf16 compute:
  A) QKV projection with SBUF-resident bf16 weights, X streamed.
  B) Flash-style causal attention in S^T form (scores computed as K^T.Q so
     exp() writes P^T directly -- no P transposes), V transposed on-chip.
  C) Per-batch AllToAll of the small pre-o_proj activations (features ->
     token shards), then a local full-width o_proj per core.
Host reassembles the token-sharded outputs.
"""

import math
import sys

for _p in ("/opt/trn_rl_repo",):
    if _p not in sys.path:
        sys.path.insert(0, _p)

import numpy as np
import ml_dtypes

import concourse.bass as bass
import concourse.mybir as mybir
import concourse.tile as tile
from concourse import bacc, bass_utils

F32 = mybir.dt.float32
F32R = mybir.dt.float32r
BF16 = mybir.dt.bfloat16
BF = ml_dtypes.bfloat16


class Cfg:
    def __init__(self, hidden=5120, n_heads=40, dh=128, B=2, S=2048, n_cores=8):
        self.hidden = hidden
        self.n_heads = n_heads
        self.dh = dh
        self.B = B
        self.S = S
        self.n_cores = n_cores
        assert dh == 128
        self.HL = n_heads // n_cores          # heads per core (5)
        self.F = 3 * self.HL * dh             # per-core packed qkv rows (1920)
        self.FO = self.HL * dh                # per-core attn feature width (640)
        self.T = B * S                        # total tokens (4096)
        self.KC = hidden // 128               # contraction chunks (40)
        self.TC = self.T // 512               # token chunks for qkv (8)
        self.SKT = S // 128                   # k tiles per batch seq (16)
        self.QC = S // 512                    # q chunks per batch (4)
        self.NFT = self.F // 128              # qkv feature tiles (15)
        self.TSH = S // n_cores               # token shard per core per batch (256)
        self.OC = hidden // 512               # o_proj out chunks (10)

    def part_heads(self):
        if self.HL > 3:
            return [(0, 3), (3, self.HL)]
        return [(0, self.HL)]

    def key(self):
        return (self.hidden, self.n_heads, self.dh, self.B, self.S, self.n_cores)


def build_program(cfg: Cfg, mode: str, dbg: bool = False):
    """mode: 'causal' (mult-mask diag blocks + block skip), 'dense' (no mask),
    'masked' (general additive mask, host passes maskT pre-scaled)."""
    assert mode in ("causal", "dense", "masked")
    c = cfg
    nc = bacc.Bacc("TRN2", target_bir_lowering=False, debug=False,
                   num_devices=c.n_cores)
    dbg_ext = None
    if dbg:
        dbg_ext = nc.dram_tensor("dbg", [128, c.HL, c.S], F32,
                                 kind="ExternalOutput").ap()

    xt = nc.dram_tensor("xt", [c.hidden, c.T], BF16, kind="ExternalInput").ap()
    wqkvt = nc.dram_tensor("wqkvt", [c.hidden, c.F], BF16,
                           kind="ExternalInput").ap()
    wot = nc.dram_tensor("wot", [c.hidden, c.hidden], BF16,
                         kind="ExternalInput").ap()
    mask_ext = None
    if mode == "masked":
        mask_ext = nc.dram_tensor("maskt", [c.S, c.S], F32,
                                  kind="ExternalInput").ap()
    # per-core output: for each batch, this core's token shard (all hidden)
    out_ext = nc.dram_tensor("out", [c.B, c.TSH, c.hidden], F32,
                             kind="ExternalOutput").ap()

    inv_sqrt_dh = 1.0 / math.sqrt(c.dh)

    xt_r = xt.rearrange("(kc p) t -> p kc t", p=128)
    wq_r = wqkvt.rearrange("(kc p) f -> p kc f", p=128)
    wo_r = wot.rearrange("(kc p) j -> p kc j", p=128)

    with tile.TileContext(nc) as tc:
        with tc.tile_pool(name="dram", bufs=1, space="DRAM") as dram:
            qkv = dram.tile([c.NFT, 128, c.T], BF16)

            # ---------------- Phase A: QKV projection -------------------
            # qkv[ft, d, t] = sum_h W[h, ft*128+d] * X[h, t]   (feature-major)
            splits = [8, 7] if c.NFT == 15 else [c.NFT]
            with tc.tile_pool(name="qkv_w", bufs=1) as wpool, \
                 tc.tile_pool(name="qkv_x", bufs=2) as xpool, \
                 tc.tile_pool(name="qkv_o", bufs=8) as opool, \
                 tc.tile_pool(name="qkv_ps", bufs=8, space="PSUM") as pspool:
                assert c.KC % 4 == 0
                KQ = c.KC // 4
                ft0 = 0
                for nft in splits:
                    wts = None
                    for tci in range(c.TC):
                        xq = [xpool.tile([128, KQ, 512], BF16, tag=f"x{j}",
                                         name=f"x{j}") for j in range(4)]
                        for j in range(4):
                            nc.sync.dma_start(
                                xq[j][:],
                                xt_r[:, j * KQ:(j + 1) * KQ,
                                     tci * 512:(tci + 1) * 512])
                        if tci == 0:
                            # per-kc weight tiles: lets the next group's
                            # weight loads overlap this group's tail
                            wts = []
                            for kc in range(c.KC):
                                w_t = wpool.tile([128, max(splits) * 128],
                                                 BF16, tag=f"w{kc}",
                                                 name=f"w{kc}")
                                nc.sync.dma_start(
                                    w_t[:, :nft * 128],
                                    wq_r[:, kc,
                                         ft0 * 128:(ft0 + nft) * 128])
                                wts.append(w_t)
                        pss = [pspool.tile([128, 512], F32, tag="ps",
                                           name=f"ps{i}")
                               for i in range(nft)]
                        for kc in range(c.KC):
                            for i in range(nft):
                                nc.tensor.matmul(
                                    pss[i][:],
                                    wts[kc][:, i * 128:(i + 1) * 128],
                                    xq[kc // KQ][:, kc % KQ, :],
                                    start=(kc == 0), stop=(kc == c.KC - 1))
                        for i in range(nft):
                            o_sb = opool.tile([128, 512], BF16, tag="o")
                            nc.vector.tensor_copy(o_sb[:], pss[i][:])
                            nc.sync.dma_start(
                                qkv[ft0 + i, :, tci * 512:(tci + 1) * 512],
                                o_sb[:])
                    ft0 += nft

            # ---------------- Phase B + C (interleaved per batch) -------
            with tc.tile_pool(name="att_const", bufs=1) as cpool, \
                 tc.tile_pool(name="att_at", bufs=2) as atpool, \
                 tc.tile_pool(name="att_in", bufs=2) as inpool, \
                 tc.tile_pool(name="att_v", bufs=1) as vpool, \
                 tc.tile_pool(name="att_pt", bufs=c.SKT + 1) as ptpool, \
                 tc.tile_pool(name="att_acc", bufs=2) as accpool, \
                 tc.tile_pool(name="att_sm", bufs=2) as smpool, \
                 tc.tile_pool(name="att_ms", bufs=(4 if mode == "masked" else 1)) as mspool, \
                 tc.tile_pool(name="op_attn", bufs=2) as apool, \
                 tc.tile_pool(name="op_w", bufs=4) as wopool, \
                 tc.tile_pool(name="op_o", bufs=2) as oopool, \
                 tc.tile_pool(name="ps_s", bufs=2, space="PSUM") as ps_s, \
                 tc.tile_pool(name="ps_at", bufs=2, space="PSUM") as ps_at, \
                 tc.tile_pool(name="ps_ms", bufs=2, space="PSUM") as ps_ms, \
                 tc.tile_pool(name="ps_op", bufs=2, space="PSUM") as ps_op:

                # constants
                ident = cpool.tile([128, 128], BF16)
                ones_col = cpool.tile([128, 1], F32R)   # lhsT for colsum
                ones_row = cpool.tile([1, 128], F32R)   # lhsT for broadcast
                with tc.tile_pool(name="att_tmp", bufs=1) as tmppool:
                    t32 = tmppool.tile([128, 128], F32)
                    nc.gpsimd.memset(t32[:], 0.0)
                    nc.gpsimd.affine_select(
                        out=t32[:], in_=t32[:],
                        compare_op=mybir.AluOpType.not_equal, fill=1.0,
                        base=0, pattern=[[-1, 128]], channel_multiplier=1)
                    nc.vector.tensor_copy(ident[:], t32[:])
                    o32 = tmppool.tile([128, 1], F32, tag="o32")
                    nc.vector.memset(o32[:], 1.0)
                    nc.vector.tensor_copy(ones_col[:], o32[:])
                    r32 = tmppool.tile([1, 128], F32, tag="r32")
                    nc.vector.memset(r32[:], 1.0)
                    nc.vector.tensor_copy(ones_row[:], r32[:])
                ctri = None
                if mode == "causal":
                    # multiplicative triangle mask [128k, 128q]:
                    # m[p, y] = 1 where y >= p else 0
                    with tc.tile_pool(name="att_cm", bufs=1) as cmtmp:
                        m32 = cmtmp.tile([128, 128], F32, tag="m32")
                        nc.gpsimd.memset(m32[:], 1.0)
                        nc.gpsimd.affine_select(
                            out=m32[:], in_=m32[:],
                            compare_op=mybir.AluOpType.is_ge, fill=0.0,
                            base=0, pattern=[[1, 128]],
                            channel_multiplier=-1)
                        ctri = cpool.tile([128, 128], BF16, tag="ctri")
                        nc.vector.tensor_copy(ctri[:], m32[:])

                pending = []

                def flush_tail():
                    # softmax tail of the previous q-chunk, emitted late so
                    # its cross-engine waits hide under the next chunk's work
                    if not pending:
                        return
                    (attnT_p, h_p, qc_p, acc_v, acc_g, at_ps, vec_only) = \
                        pending.pop()
                    if not vec_only:
                        nc.vector.tensor_tensor(acc_v[:], acc_v[:], acc_g[:],
                                                mybir.AluOpType.add)
                    den_ps = ps_ms.tile([1, 512], F32, tag="ms")
                    nc.tensor.matmul(den_ps[:], ones_col[:], acc_v[:],
                                     start=True, stop=True)
                    rden = smpool.tile([1, 512], F32, tag="rden")
                    nc.vector.reciprocal_approx_fast(rden[:], den_ps[:])
                    den_r = smpool.tile([1, 512], F32R, tag="denr")
                    nc.vector.tensor_copy(den_r[:], rden[:])
                    bc_ps = ps_ms.tile([128, 512], F32, tag="ms")
                    nc.tensor.matmul(bc_ps[:], ones_row[:], den_r[:],
                                     start=True, stop=True)
                    bc_sb = smpool.tile([128, 512], F32, tag="bc")
                    nc.vector.tensor_copy(bc_sb[:], bc_ps[:])
                    nc.vector.tensor_tensor(
                        attnT_p[:, h_p, qc_p * 512:(qc_p + 1) * 512],
                        at_ps[:], bc_sb[:], mybir.AluOpType.mult)

                def attend_head(b, h, attnT, acc_vec_only=False):
                    t0 = b * c.S
                    q_sb = inpool.tile([128, c.S], BF16, tag="q")
                    k_sb = inpool.tile([128, c.S], BF16, tag="k")
                    v_sb = inpool.tile([128, c.S], BF16, tag="v")
                    nc.sync.dma_start(q_sb[:], qkv[h, :, t0:t0 + c.S])
                    nc.sync.dma_start(k_sb[:], qkv[c.HL + h, :, t0:t0 + c.S])
                    nc.sync.dma_start(v_sb[:],
                                      qkv[2 * c.HL + h, :, t0:t0 + c.S])
                    v_tok = None

                    for qc in range(c.QC):
                        nkt = 4 * (qc + 1) if mode == "causal" else c.SKT
                        acc_g = accpool.tile([128, 512], F32R, tag="accg")
                        acc_v = accpool.tile([128, 512], F32R, tag="accv")
                        pts = []
                        for kt in range(nkt):
                            off = kt - 4 * qc  # >=0: diagonal tile (causal)
                            pt = ptpool.tile([128, 512], BF16, tag="pt")
                            s_ps = ps_s.tile([128, 512], F32, tag="s")
                            if mode == "causal" and off > 0:
                                # valid q range is [off*128, 512)
                                w = 512 - off * 128
                                nc.tensor.matmul(
                                    s_ps[:, :w],
                                    k_sb[:, kt * 128:(kt + 1) * 128],
                                    q_sb[:, qc * 512 + off * 128:
                                         (qc + 1) * 512],
                                    start=True, stop=True)
                                nc.vector.memset(pt[:, :off * 128], 0.0)
                                nc.scalar.activation(
                                    pt[:, off * 128:], s_ps[:, :w],
                                    mybir.ActivationFunctionType.Exp,
                                    scale=inv_sqrt_dh)
                                nc.vector.tensor_tensor(
                                    pt[:, off * 128:(off + 1) * 128],
                                    pt[:, off * 128:(off + 1) * 128],
                                    ctri[:], mybir.AluOpType.mult)
                            else:
                                nc.tensor.matmul(
                                    s_ps[:],
                                    k_sb[:, kt * 128:(kt + 1) * 128],
                                    q_sb[:, qc * 512:(qc + 1) * 512],
                                    start=True, stop=True)
                                if mode == "masked":
                                    m_sb = mspool.tile([128, 512], F32,
                                                       tag="m")
                                    nc.sync.dma_start(
                                        m_sb[:],
                                        mask_ext[kt * 128:(kt + 1) * 128,
                                                 qc * 512:(qc + 1) * 512])
                                    nc.vector.tensor_tensor(
                                        s_ps[:], s_ps[:], m_sb[:],
                                        mybir.AluOpType.add)
                                nc.scalar.activation(
                                    pt[:], s_ps[:],
                                    mybir.ActivationFunctionType.Exp,
                                    scale=inv_sqrt_dh)
                                if mode == "causal" and off == 0:
                                    nc.vector.tensor_tensor(
                                        pt[:, :128], pt[:, :128],
                                        ctri[:], mybir.AluOpType.mult)
                            # denominator: two parallel accumulation chains
                            if kt % 2 == 0 and not acc_vec_only:
                                if kt == 0:
                                    nc.gpsimd.tensor_copy(acc_g[:], pt[:])
                                else:
                                    nc.gpsimd.tensor_tensor(
                                        acc_g[:], acc_g[:], pt[:],
                                        mybir.AluOpType.add)
                            else:
                                if kt == (0 if acc_vec_only else 1):
                                    nc.vector.tensor_copy(acc_v[:], pt[:])
                                else:
                                    nc.vector.tensor_tensor(
                                        acc_v[:], acc_v[:], pt[:],
                                        mybir.AluOpType.add)
                            pts.append(pt)
                        if qc == 0:
                            # V to token-major [tok_p, st, dh]; deferred past
                            # qc0's QK so the head start doesn't stall on the
                            # v_sb DMA
                            v_tok = vpool.tile([128, c.SKT, 128], BF16,
                                               tag="vt")
                            for st in range(c.SKT):
                                vt_ps = ps_ms.tile([128, 128], BF16,
                                                   tag="ms")
                                nc.tensor.matmul(
                                    vt_ps[:],
                                    v_sb[:, st * 128:(st + 1) * 128],
                                    ident[:], is_transpose=True)
                                nc.vector.tensor_copy(v_tok[:, st, :],
                                                      vt_ps[:])
                        # PV; diagonal tiles only contribute to their valid
                        # q range (the rest of pt is zero), so slice them
                        at_ps = ps_at.tile([128, 512], F32, tag="at")
                        for kt in range(nkt):
                            off = kt - 4 * qc
                            q0 = off * 128 if (mode == "causal" and off > 0) \
                                else 0
                            nc.tensor.matmul(
                                at_ps[:, q0:], v_tok[:, kt, :],
                                pts[kt][:, q0:],
                                start=(kt == 0), stop=(kt == nkt - 1))
                        # softmax tail of the PREVIOUS chunk, now that its
                        # inputs are long ready; ours is deferred
                        flush_tail()
                        pending.append((attnT, h, qc, acc_v, acc_g, at_ps,
                                        acc_vec_only))

                # heads split into parts so part 0's AllToAll can launch
                # after its heads finish, overlapping later heads' attention
                part_heads = c.part_heads()

                def a2a_part(b, attnT, p):
                    flush_tail()
                    h0, h1 = part_heads[p]
                    nh = h1 - h0
                    a2a_in = dram.tile([c.n_cores, nh * 128, c.TSH], BF16,
                                       tag=f"a2a_in{b}{p}",
                                       name=f"a2a_in{b}{p}")
                    a2a_out = dram.tile([c.n_cores, nh * 128, c.TSH], BF16,
                                        tag=f"a2a_out{b}{p}",
                                        name=f"a2a_out{b}{p}")
                    # issued from gpsimd: keeps the sync queue free for the
                    # next head's input prefetch (the CC waits on these on
                    # the same queue anyway)
                    for g in range(c.n_cores):
                        nc.gpsimd.dma_start(
                            a2a_in[g].rearrange("(f q) t -> q f t", q=128),
                            attnT[:, h0:h1, g * c.TSH:(g + 1) * c.TSH])
                    nc.gpsimd.collective_compute(
                        "AllToAll",
                        mybir.AluOpType.bypass,
                        replica_groups=[list(range(c.n_cores))],
                        ins=[a2a_in[:].opt()],
                        outs=[a2a_out[:].opt()],
                    )
                    # gathered: [n_cores*nh*128 feats, TSH tokens]; issued
                    # from gpsimd so the collective wait stays off the
                    # DMA-prefetch queue
                    attn_sb = apool.tile([128, c.n_cores * nh, c.TSH], BF16,
                                         tag=f"ag{p}", name=f"ag{b}{p}")
                    nc.gpsimd.dma_start(
                        attn_sb[:],
                        a2a_out.rearrange("s (f q) t -> q (s f) t", q=128))
                    return attn_sb

                # per-oc contraction layout: (part, fc offset in part, count)
                wo_layout = []
                fc0 = 0
                for p, (h0, h1) in enumerate(part_heads):
                    nfc = c.n_cores * (h1 - h0)
                    ka = nfc // 2
                    wo_layout.append((p, 0, ka, fc0))
                    wo_layout.append((p, ka, nfc - ka, fc0 + ka))
                    fc0 += nfc

                max_cnt = max(le[2] for le in wo_layout)

                def o_proj_chunk(b, parts, oc):
                    wo_sbs = []
                    for (p, k0, cnt, gfc) in wo_layout:
                        wo_t = wopool.tile([128, max_cnt, 512], BF16,
                                           tag="wo")
                        nc.sync.dma_start(
                            wo_t[:, :cnt, :],
                            wo_r[:, gfc:gfc + cnt,
                                 oc * 512:(oc + 1) * 512])
                        wo_sbs.append(wo_t)
                    last = len(wo_layout) - 1
                    for tt in range(c.TSH // 128):
                        ps = ps_op.tile([128, 512], F32, tag="ops")
                        for wi, (p, k0, cnt, gfc) in enumerate(wo_layout):
                            for k in range(cnt):
                                nc.tensor.matmul(
                                    ps[:],
                                    parts[p][:, k0 + k,
                                             tt * 128:(tt + 1) * 128],
                                    wo_sbs[wi][:, k, :],
                                    start=(wi == 0 and k == 0),
                                    stop=(wi == last and k == cnt - 1))
                        po_sb = oopool.tile([128, 512], F32, tag="po")
                        nc.vector.tensor_copy(po_sb[:], ps[:])
                        nc.gpsimd.dma_start(
                            out_ext[b, tt * 128:(tt + 1) * 128,
                                    oc * 512:(oc + 1) * 512],
                            po_sb[:])

                # batch 0: attention; part-0 AllToAll launches mid-batch
                two_parts = len(part_heads) > 1
                h_p0 = part_heads[0][1] - 1
                parts0 = [None] * len(part_heads)
                parts1 = [None] * len(part_heads)
                attnT0 = atpool.tile([128, c.HL, c.S], BF16, tag="attnT",
                                     name="attnT0")
                for h in range(c.HL):
                    # the head after the part-0 collective launch keeps its
                    # denominator chains off gpsimd (busy running the CC)
                    attend_head(0, h, attnT0,
                                acc_vec_only=(two_parts and h == h_p0 + 1))
                    if two_parts and h == h_p0:
                        parts0[0] = a2a_part(0, attnT0, 0)
                if dbg:
                    flush_tail()
                    dbg_sb = smpool.tile([128, c.S], F32, tag="dbg")
                    for hh in range(c.HL):
                        nc.vector.tensor_copy(dbg_sb[:], attnT0[:, hh, :])
                        nc.sync.dma_start(dbg_ext[:, hh, :], dbg_sb[:])
                attnT1 = atpool.tile([128, c.HL, c.S], BF16, tag="attnT",
                                     name="attnT1")
                # batch 1 attention interleaved with batch 0 o_proj chunks
                attend_head(1, 0, attnT1, acc_vec_only=True)
                if two_parts:
                    parts0[1] = a2a_part(0, attnT0, 1)
                else:
                    parts0[0] = a2a_part(0, attnT0, 0)
                done = 0
                for h in range(1, c.HL):
                    attend_head(1, h, attnT1,
                                acc_vec_only=(h == 1 or
                                              (two_parts and h == h_p0 + 1)))
                    if two_parts and h == h_p0:
                        parts1[0] = a2a_part(1, attnT1, 0)
                    tgt = (c.OC * h) // (c.HL - 1)
                    while done < tgt:
                        o_proj_chunk(0, parts0, done)
                        done += 1
                while done < c.OC:
                    o_proj_chunk(0, parts0, done)
                    done += 1
                if two_parts:
                    parts1[1] = a2a_part(1, attnT1, 1)
                else:
                    parts1[0] = a2a_part(1, attnT1, 0)
                for oc in range(c.OC):
                    o_proj_chunk(1, parts1, oc)

    nc.compile()
    return nc


# --------------------------------------------------------------------------
_CACHE = {}


def _get_program(cfg: Cfg, mode: str):
    key = (cfg.key(), mode)
    if key not in _CACHE:
        _CACHE[key] = build_program(cfg, mode)
    return _CACHE[key]


def prepare_inputs(cfg: Cfg, hidden_states, attention_mask, W_pack, W_o):
    """Host-side shard + layout prep (bf16 cast). Returns (mode, in_maps)."""
    c = cfg
    X = np.asarray(hidden_states, dtype=np.float32).reshape(c.T, c.hidden)
    XT = np.ascontiguousarray(X.T).astype(BF)

    mask = np.asarray(attention_mask, dtype=np.float32).reshape(c.S, c.S)
    causal_ref = np.where(
        np.tril(np.ones((c.S, c.S), dtype=bool)), 0.0, -1e9
    ).astype(np.float32)
    if np.array_equal(mask, causal_ref):
        mode = "causal"
    elif not mask.any():
        mode = "dense"
    else:
        mode = "masked"

    W_pack = np.asarray(W_pack, dtype=np.float32)
    W_o = np.asarray(W_o, dtype=np.float32)
    H = c.hidden
    # woT rows (features) reordered to the part-concatenated gather order:
    # for each head part, src-core-major then local head
    order = [s * c.HL + j
             for (h0, h1) in c.part_heads()
             for s in range(c.n_cores)
             for j in range(h0, h1)]
    woT = np.ascontiguousarray(
        W_o.T.reshape(c.n_heads, c.dh, c.hidden)[order]
        .reshape(c.hidden, c.hidden)).astype(BF)   # [feat, out] full
    in_maps = []
    for g in range(c.n_cores):
        r0, r1 = g * c.FO, (g + 1) * c.FO
        wq = W_pack[r0:r1]
        wk = W_pack[H + r0:H + r1]
        wv = W_pack[2 * H + r0:2 * H + r1]
        wqkvT = np.ascontiguousarray(
            np.concatenate([wq, wk, wv], axis=0).T).astype(BF)  # [H, F]
        m = {"xt": XT, "wqkvt": wqkvT, "wot": woT}
        if mode == "masked":
            m["maskt"] = np.ascontiguousarray(mask.T * math.sqrt(c.dh))
        in_maps.append(m)
    return mode, in_maps


def assemble_output(cfg: Cfg, results):
    c = cfg
    full = np.empty((c.B, c.S, c.hidden), dtype=np.float32)
    for g in range(c.n_cores):
        o = results[g]["out"].reshape(c.B, c.TSH, c.hidden)
        for b in range(c.B):
            full[b, g * c.TSH:(g + 1) * c.TSH] = o[b]
    return full


def kernel(hidden_states, attention_mask, W_pack, W_o):
    cfg = Cfg()
    mode, in_maps = prepare_inputs(cfg, hidden_states, attention_mask,
                                   W_pack, W_o)
    nc = _get_program(cfg, mode)
    res = bass_utils.run_bass_kernel_spmd(nc, in_maps,
                                          list(range(cfg.n_cores)))
    return assemble_output(cfg, res.results)


# revision 52
# speedup vs baseline: 1.0038x; 1.0038x over previous
"""Trainium2 Bass kernel for BaichuanAttention (hidden=5120, 40 heads, b=2, s=2048).

Tensor-parallel over heads across 8 NeuronCores, bf16 compute:
  A) QKV projection with SBUF-resident bf16 weights, X streamed.
  B) Flash-style causal attention in S^T form (scores computed as K^T.Q so
     exp() writes P^T directly -- no P transposes), V transposed on-chip.
  C) Per-batch AllToAll of the small pre-o_proj activations (features ->
     token shards), then a local full-width o_proj per core.
Host reassembles the token-sharded outputs.
"""

import math
import sys

for _p in ("/opt/trn_rl_repo",):
    if _p not in sys.path:
        sys.path.insert(0, _p)

import numpy as np
import ml_dtypes

import concourse.bass as bass
import concourse.mybir as mybir
import concourse.tile as tile
from concourse import bacc, bass_utils

F32 = mybir.dt.float32
F32R = mybir.dt.float32r
BF16 = mybir.dt.bfloat16
BF = ml_dtypes.bfloat16


class Cfg:
    def __init__(self, hidden=5120, n_heads=40, dh=128, B=2, S=2048, n_cores=8):
        self.hidden = hidden
        self.n_heads = n_heads
        self.dh = dh
        self.B = B
        self.S = S
        self.n_cores = n_cores
        assert dh == 128
        self.HL = n_heads // n_cores          # heads per core (5)
        self.F = 3 * self.HL * dh             # per-core packed qkv rows (1920)
        self.FO = self.HL * dh                # per-core attn feature width (640)
        self.T = B * S                        # total tokens (4096)
        self.KC = hidden // 128               # contraction chunks (40)
        self.TC = self.T // 512               # token chunks for qkv (8)
        self.SKT = S // 128                   # k tiles per batch seq (16)
        self.QC = S // 512                    # q chunks per batch (4)
        self.NFT = self.F // 128              # qkv feature tiles (15)
        self.TSH = S // n_cores               # token shard per core per batch (256)
        self.OC = hidden // 512               # o_proj out chunks (10)

    def part_heads(self):
        if self.HL > 3:
            return [(0, 3), (3, self.HL)]
        return [(0, self.HL)]

    def key(self):
        return (self.hidden, self.n_heads, self.dh, self.B, self.S, self.n_cores)


def build_program(cfg: Cfg, mode: str, dbg: bool = False):
    """mode: 'causal' (mult-mask diag blocks + block skip), 'dense' (no mask),
    'masked' (general additive mask, host passes maskT pre-scaled)."""
    assert mode in ("causal", "dense", "masked")
    c = cfg
    nc = bacc.Bacc("TRN2", target_bir_lowering=False, debug=False,
                   num_devices=c.n_cores)
    dbg_ext = None
    if dbg:
        dbg_ext = nc.dram_tensor("dbg", [128, c.HL, c.S], F32,
                                 kind="ExternalOutput").ap()

    xt = nc.dram_tensor("xt", [c.hidden, c.T], BF16, kind="ExternalInput").ap()
    wqkvt = nc.dram_tensor("wqkvt", [c.hidden, c.F], BF16,
                           kind="ExternalInput").ap()
    wot = nc.dram_tensor("wot", [c.hidden, c.hidden], BF16,
                         kind="ExternalInput").ap()
    mask_ext = None
    if mode == "masked":
        mask_ext = nc.dram_tensor("maskt", [c.S, c.S], F32,
                                  kind="ExternalInput").ap()
    # per-core output: for each batch, this core's token shard (all hidden)
    out_ext = nc.dram_tensor("out", [c.B, c.TSH, c.hidden], F32,
                             kind="ExternalOutput").ap()

    inv_sqrt_dh = 1.0 / math.sqrt(c.dh)

    xt_r = xt.rearrange("(kc p) t -> p kc t", p=128)
    wq_r = wqkvt.rearrange("(kc p) f -> p kc f", p=128)
    wo_r = wot.rearrange("(kc p) j -> p kc j", p=128)

    with tile.TileContext(nc) as tc:
        with tc.tile_pool(name="dram", bufs=1, space="DRAM") as dram:
            qkv = dram.tile([c.NFT, 128, c.T], BF16)

            # ---------------- Phase A: QKV projection -------------------
            # qkv[ft, d, t] = sum_h W[h, ft*128+d] * X[h, t]   (feature-major)
            splits = [8, 7] if c.NFT == 15 else [c.NFT]
            with tc.tile_pool(name="qkv_w", bufs=1) as wpool, \
                 tc.tile_pool(name="qkv_x", bufs=2) as xpool, \
                 tc.tile_pool(name="qkv_o", bufs=8) as opool, \
                 tc.tile_pool(name="qkv_ps", bufs=8, space="PSUM") as pspool:
                assert c.KC % 4 == 0
                KQ = c.KC // 4
                ft0 = 0
                for nft in splits:
                    wts = None
                    for tci in range(c.TC):
                        xq = [xpool.tile([128, KQ, 512], BF16, tag=f"x{j}",
                                         name=f"x{j}") for j in range(4)]
                        for j in range(4):
                            nc.sync.dma_start(
                                xq[j][:],
                                xt_r[:, j * KQ:(j + 1) * KQ,
                                     tci * 512:(tci + 1) * 512])
                        if tci == 0:
                            # per-kc weight tiles: lets the next group's
                            # weight loads overlap this group's tail
                            wts = []
                            for kc in range(c.KC):
                                w_t = wpool.tile([128, max(splits) * 128],
                                                 BF16, tag=f"w{kc}",
                                                 name=f"w{kc}")
                                nc.sync.dma_start(
                                    w_t[:, :nft * 128],
                                    wq_r[:, kc,
                                         ft0 * 128:(ft0 + nft) * 128])
                                wts.append(w_t)
                        pss = [pspool.tile([128, 512], F32, tag="ps",
                                           name=f"ps{i}")
                               for i in range(nft)]
                        for kc in range(c.KC):
                            for i in range(nft):
                                nc.tensor.matmul(
                                    pss[i][:],
                                    wts[kc][:, i * 128:(i + 1) * 128],
                                    xq[kc // KQ][:, kc % KQ, :],
                                    start=(kc == 0), stop=(kc == c.KC - 1))
                        for i in range(nft):
                            o_sb = opool.tile([128, 512], BF16, tag="o")
                            nc.vector.tensor_copy(o_sb[:], pss[i][:])
                            nc.sync.dma_start(
                                qkv[ft0 + i, :, tci * 512:(tci + 1) * 512],
                                o_sb[:])
                    ft0 += nft

            # ---------------- Phase B + C (interleaved per batch) -------
            with tc.tile_pool(name="att_const", bufs=1) as cpool, \
                 tc.tile_pool(name="att_at", bufs=2) as atpool, \
                 tc.tile_pool(name="att_in", bufs=3) as inpool, \
                 tc.tile_pool(name="att_v", bufs=1) as vpool, \
                 tc.tile_pool(name="att_pt", bufs=c.SKT + 1) as ptpool, \
                 tc.tile_pool(name="att_acc", bufs=2) as accpool, \
                 tc.tile_pool(name="att_sm", bufs=2) as smpool, \
                 tc.tile_pool(name="att_ms", bufs=(4 if mode == "masked" else 1)) as mspool, \
                 tc.tile_pool(name="op_attn", bufs=1) as apool, \
                 tc.tile_pool(name="op_w", bufs=4) as wopool, \
                 tc.tile_pool(name="op_o", bufs=2) as oopool, \
                 tc.tile_pool(name="ps_s", bufs=2, space="PSUM") as ps_s, \
                 tc.tile_pool(name="ps_at", bufs=2, space="PSUM") as ps_at, \
                 tc.tile_pool(name="ps_ms", bufs=2, space="PSUM") as ps_ms, \
                 tc.tile_pool(name="ps_op", bufs=2, space="PSUM") as ps_op:

                # constants
                ident = cpool.tile([128, 128], BF16)
                ones_col = cpool.tile([128, 1], F32R)   # lhsT for colsum
                ones_row = cpool.tile([1, 128], F32R)   # lhsT for broadcast
                with tc.tile_pool(name="att_tmp", bufs=1) as tmppool:
                    t32 = tmppool.tile([128, 128], F32)
                    nc.gpsimd.memset(t32[:], 0.0)
                    nc.gpsimd.affine_select(
                        out=t32[:], in_=t32[:],
                        compare_op=mybir.AluOpType.not_equal, fill=1.0,
                        base=0, pattern=[[-1, 128]], channel_multiplier=1)
                    nc.vector.tensor_copy(ident[:], t32[:])
                    o32 = tmppool.tile([128, 1], F32, tag="o32")
                    nc.vector.memset(o32[:], 1.0)
                    nc.vector.tensor_copy(ones_col[:], o32[:])
                    r32 = tmppool.tile([1, 128], F32, tag="r32")
                    nc.vector.memset(r32[:], 1.0)
                    nc.vector.tensor_copy(ones_row[:], r32[:])
                ctri = None
                if mode == "causal":
                    # multiplicative triangle mask [128k, 128q]:
                    # m[p, y] = 1 where y >= p else 0
                    with tc.tile_pool(name="att_cm", bufs=1) as cmtmp:
                        m32 = cmtmp.tile([128, 128], F32, tag="m32")
                        nc.gpsimd.memset(m32[:], 1.0)
                        nc.gpsimd.affine_select(
                            out=m32[:], in_=m32[:],
                            compare_op=mybir.AluOpType.is_ge, fill=0.0,
                            base=0, pattern=[[1, 128]],
                            channel_multiplier=-1)
                        ctri = cpool.tile([128, 128], BF16, tag="ctri")
                        nc.vector.tensor_copy(ctri[:], m32[:])

                pending = []

                def flush_tail():
                    # softmax tail of the previous q-chunk, emitted late so
                    # its cross-engine waits hide under the next chunk's work
                    if not pending:
                        return
                    (attnT_p, h_p, qc_p, acc_v, acc_g, at_ps, vec_only) = \
                        pending.pop()
                    if not vec_only:
                        nc.vector.tensor_tensor(acc_v[:], acc_v[:], acc_g[:],
                                                mybir.AluOpType.add)
                    den_ps = ps_ms.tile([1, 512], F32, tag="ms")
                    nc.tensor.matmul(den_ps[:], ones_col[:], acc_v[:],
                                     start=True, stop=True)
                    rden = smpool.tile([1, 512], F32, tag="rden")
                    nc.vector.reciprocal_approx_fast(rden[:], den_ps[:])
                    den_r = smpool.tile([1, 512], F32R, tag="denr")
                    nc.vector.tensor_copy(den_r[:], rden[:])
                    bc_ps = ps_ms.tile([128, 512], F32, tag="ms")
                    nc.tensor.matmul(bc_ps[:], ones_row[:], den_r[:],
                                     start=True, stop=True)
                    bc_sb = smpool.tile([128, 512], F32, tag="bc")
                    nc.vector.tensor_copy(bc_sb[:], bc_ps[:])
                    nc.vector.tensor_tensor(
                        attnT_p[:, h_p, qc_p * 512:(qc_p + 1) * 512],
                        at_ps[:], bc_sb[:], mybir.AluOpType.mult)

                def attend_head(b, h, attnT, acc_vec_only=False):
                    t0 = b * c.S
                    q_sb = inpool.tile([128, c.S], BF16, tag="q")
                    k_sb = inpool.tile([128, c.S], BF16, tag="k")
                    v_sb = inpool.tile([128, c.S], BF16, tag="v")
                    nc.sync.dma_start(q_sb[:], qkv[h, :, t0:t0 + c.S])
                    nc.sync.dma_start(k_sb[:], qkv[c.HL + h, :, t0:t0 + c.S])
                    nc.sync.dma_start(v_sb[:],
                                      qkv[2 * c.HL + h, :, t0:t0 + c.S])
                    v_tok = None

                    for qc in range(c.QC):
                        nkt = 4 * (qc + 1) if mode == "causal" else c.SKT
                        acc_g = accpool.tile([128, 512], F32R, tag="accg")
                        acc_v = accpool.tile([128, 512], F32R, tag="accv")
                        pts = []
                        for kt in range(nkt):
                            off = kt - 4 * qc  # >=0: diagonal tile (causal)
                            pt = ptpool.tile([128, 512], BF16, tag="pt")
                            s_ps = ps_s.tile([128, 512], F32, tag="s")
                            if mode == "causal" and off > 0:
                                # valid q range is [off*128, 512)
                                w = 512 - off * 128
                                nc.tensor.matmul(
                                    s_ps[:, :w],
                                    k_sb[:, kt * 128:(kt + 1) * 128],
                                    q_sb[:, qc * 512 + off * 128:
                                         (qc + 1) * 512],
                                    start=True, stop=True)
                                nc.vector.memset(pt[:, :off * 128], 0.0)
                                nc.scalar.activation(
                                    pt[:, off * 128:], s_ps[:, :w],
                                    mybir.ActivationFunctionType.Exp,
                                    scale=inv_sqrt_dh)
                                nc.vector.tensor_tensor(
                                    pt[:, off * 128:(off + 1) * 128],
                                    pt[:, off * 128:(off + 1) * 128],
                                    ctri[:], mybir.AluOpType.mult)
                            else:
                                nc.tensor.matmul(
                                    s_ps[:],
                                    k_sb[:, kt * 128:(kt + 1) * 128],
                                    q_sb[:, qc * 512:(qc + 1) * 512],
                                    start=True, stop=True)
                                if mode == "masked":
                                    m_sb = mspool.tile([128, 512], F32,
                                                       tag="m")
                                    nc.sync.dma_start(
                                        m_sb[:],
                                        mask_ext[kt * 128:(kt + 1) * 128,
                                                 qc * 512:(qc + 1) * 512])
                                    nc.vector.tensor_tensor(
                                        s_ps[:], s_ps[:], m_sb[:],
                                        mybir.AluOpType.add)
                                nc.scalar.activation(
                                    pt[:], s_ps[:],
                                    mybir.ActivationFunctionType.Exp,
                                    scale=inv_sqrt_dh)
                                if mode == "causal" and off == 0:
                                    nc.vector.tensor_tensor(
                                        pt[:, :128], pt[:, :128],
                                        ctri[:], mybir.AluOpType.mult)
                            # denominator: two parallel accumulation chains
                            if kt % 2 == 0 and not acc_vec_only:
                                if kt == 0:
                                    nc.gpsimd.tensor_copy(acc_g[:], pt[:])
                                else:
                                    nc.gpsimd.tensor_tensor(
                                        acc_g[:], acc_g[:], pt[:],
                                        mybir.AluOpType.add)
                            else:
                                if kt == (0 if acc_vec_only else 1):
                                    nc.vector.tensor_copy(acc_v[:], pt[:])
                                else:
                                    nc.vector.tensor_tensor(
                                        acc_v[:], acc_v[:], pt[:],
                                        mybir.AluOpType.add)
                            pts.append(pt)
                        if qc == 0:
                            # V to token-major [tok_p, st, dh]; deferred past
                            # qc0's QK so the head start doesn't stall on the
                            # v_sb DMA
                            v_tok = vpool.tile([128, c.SKT, 128], BF16,
                                               tag="vt")
                            for st in range(c.SKT):
                                vt_ps = ps_ms.tile([128, 128], BF16,
                                                   tag="ms")
                                nc.tensor.matmul(
                                    vt_ps[:],
                                    v_sb[:, st * 128:(st + 1) * 128],
                                    ident[:], is_transpose=True)
                                nc.vector.tensor_copy(v_tok[:, st, :],
                                                      vt_ps[:])
                        # PV; diagonal tiles only contribute to their valid
                        # q range (the rest of pt is zero), so slice them
                        at_ps = ps_at.tile([128, 512], F32, tag="at")
                        for kt in range(nkt):
                            off = kt - 4 * qc
                            q0 = off * 128 if (mode == "causal" and off > 0) \
                                else 0
                            nc.tensor.matmul(
                                at_ps[:, q0:], v_tok[:, kt, :],
                                pts[kt][:, q0:],
                                start=(kt == 0), stop=(kt == nkt - 1))
                        # softmax tail of the PREVIOUS chunk, now that its
                        # inputs are long ready; ours is deferred
                        flush_tail()
                        pending.append((attnT, h, qc, acc_v, acc_g, at_ps,
                                        acc_vec_only))

                # heads split into parts so part 0's AllToAll can launch
                # after its heads finish, overlapping later heads' attention
                part_heads = c.part_heads()

                def a2a_part(b, attnT, p):
                    flush_tail()
                    h0, h1 = part_heads[p]
                    nh = h1 - h0
                    a2a_in = dram.tile([c.n_cores, nh * 128, c.TSH], BF16,
                                       tag=f"a2a_in{b}{p}",
                                       name=f"a2a_in{b}{p}")
                    a2a_out = dram.tile([c.n_cores, nh * 128, c.TSH], BF16,
                                        tag=f"a2a_out{b}{p}",
                                        name=f"a2a_out{b}{p}")
                    # issued from gpsimd: keeps the sync queue free for the
                    # next head's input prefetch (the CC waits on these on
                    # the same queue anyway)
                    for g in range(c.n_cores):
                        nc.gpsimd.dma_start(
                            a2a_in[g].rearrange("(f q) t -> q f t", q=128),
                            attnT[:, h0:h1, g * c.TSH:(g + 1) * c.TSH])
                    nc.gpsimd.collective_compute(
                        "AllToAll",
                        mybir.AluOpType.bypass,
                        replica_groups=[list(range(c.n_cores))],
                        ins=[a2a_in[:].opt()],
                        outs=[a2a_out[:].opt()],
                    )
                    # gathered: [n_cores*nh*128 feats, TSH tokens]; issued
                    # from gpsimd so the collective wait stays off the
                    # DMA-prefetch queue
                    attn_sb = apool.tile([128, c.n_cores * nh, c.TSH], BF16,
                                         tag=f"ag{p}", name=f"ag{b}{p}")
                    nc.gpsimd.dma_start(
                        attn_sb[:],
                        a2a_out.rearrange("s (f q) t -> q (s f) t", q=128))
                    return attn_sb

                # per-oc contraction layout: (part, fc offset in part, count)
                wo_layout = []
                fc0 = 0
                for p, (h0, h1) in enumerate(part_heads):
                    nfc = c.n_cores * (h1 - h0)
                    ka = nfc // 2
                    wo_layout.append((p, 0, ka, fc0))
                    wo_layout.append((p, ka, nfc - ka, fc0 + ka))
                    fc0 += nfc

                max_cnt = max(le[2] for le in wo_layout)

                def o_proj_chunk(b, parts, oc):
                    wo_sbs = []
                    for (p, k0, cnt, gfc) in wo_layout:
                        wo_t = wopool.tile([128, max_cnt, 512], BF16,
                                           tag="wo")
                        nc.sync.dma_start(
                            wo_t[:, :cnt, :],
                            wo_r[:, gfc:gfc + cnt,
                                 oc * 512:(oc + 1) * 512])
                        wo_sbs.append(wo_t)
                    last = len(wo_layout) - 1
                    for tt in range(c.TSH // 128):
                        ps = ps_op.tile([128, 512], F32, tag="ops")
                        for wi, (p, k0, cnt, gfc) in enumerate(wo_layout):
                            for k in range(cnt):
                                nc.tensor.matmul(
                                    ps[:],
                                    parts[p][:, k0 + k,
                                             tt * 128:(tt + 1) * 128],
                                    wo_sbs[wi][:, k, :],
                                    start=(wi == 0 and k == 0),
                                    stop=(wi == last and k == cnt - 1))
                        po_sb = oopool.tile([128, 512], F32, tag="po")
                        nc.vector.tensor_copy(po_sb[:], ps[:])
                        nc.gpsimd.dma_start(
                            out_ext[b, tt * 128:(tt + 1) * 128,
                                    oc * 512:(oc + 1) * 512],
                            po_sb[:])

                # batch 0: attention; part-0 AllToAll launches mid-batch
                two_parts = len(part_heads) > 1
                h_p0 = part_heads[0][1] - 1
                parts0 = [None] * len(part_heads)
                parts1 = [None] * len(part_heads)
                attnT0 = atpool.tile([128, c.HL, c.S], BF16, tag="attnT",
                                     name="attnT0")
                for h in range(c.HL):
                    # the head after the part-0 collective launch keeps its
                    # denominator chains off gpsimd (busy running the CC)
                    attend_head(0, h, attnT0,
                                acc_vec_only=(two_parts and h == h_p0 + 1))
                    if two_parts and h == h_p0:
                        parts0[0] = a2a_part(0, attnT0, 0)
                if dbg:
                    flush_tail()
                    dbg_sb = smpool.tile([128, c.S], F32, tag="dbg")
                    for hh in range(c.HL):
                        nc.vector.tensor_copy(dbg_sb[:], attnT0[:, hh, :])
                        nc.sync.dma_start(dbg_ext[:, hh, :], dbg_sb[:])
                attnT1 = atpool.tile([128, c.HL, c.S], BF16, tag="attnT",
                                     name="attnT1")
                # batch 1 attention interleaved with batch 0 o_proj chunks
                attend_head(1, 0, attnT1, acc_vec_only=True)
                if two_parts:
                    parts0[1] = a2a_part(0, attnT0, 1)
                else:
                    parts0[0] = a2a_part(0, attnT0, 0)
                done = 0
                for h in range(1, c.HL):
                    attend_head(1, h, attnT1,
                                acc_vec_only=(h == 1 or
                                              (two_parts and h == h_p0 + 1)))
                    if two_parts and h == h_p0:
                        parts1[0] = a2a_part(1, attnT1, 0)
                    tgt = (c.OC * h) // (c.HL - 1)
                    while done < tgt:
                        o_proj_chunk(0, parts0, done)
                        done += 1
                while done < c.OC:
                    o_proj_chunk(0, parts0, done)
                    done += 1
                if two_parts:
                    parts1[1] = a2a_part(1, attnT1, 1)
                else:
                    parts1[0] = a2a_part(1, attnT1, 0)
                for oc in range(c.OC):
                    o_proj_chunk(1, parts1, oc)

    nc.compile()
    return nc


# --------------------------------------------------------------------------
_CACHE = {}


def _get_program(cfg: Cfg, mode: str):
    key = (cfg.key(), mode)
    if key not in _CACHE:
        _CACHE[key] = build_program(cfg, mode)
    return _CACHE[key]


def prepare_inputs(cfg: Cfg, hidden_states, attention_mask, W_pack, W_o):
    """Host-side shard + layout prep (bf16 cast). Returns (mode, in_maps)."""
    c = cfg
    X = np.asarray(hidden_states, dtype=np.float32).reshape(c.T, c.hidden)
    XT = np.ascontiguousarray(X.T).astype(BF)

    mask = np.asarray(attention_mask, dtype=np.float32).reshape(c.S, c.S)
    causal_ref = np.where(
        np.tril(np.ones((c.S, c.S), dtype=bool)), 0.0, -1e9
    ).astype(np.float32)
    if np.array_equal(mask, causal_ref):
        mode = "causal"
    elif not mask.any():
        mode = "dense"
    else:
        mode = "masked"

    W_pack = np.asarray(W_pack, dtype=np.float32)
    W_o = np.asarray(W_o, dtype=np.float32)
    H = c.hidden
    # woT rows (features) reordered to the part-concatenated gather order:
    # for each head part, src-core-major then local head
    order = [s * c.HL + j
             for (h0, h1) in c.part_heads()
             for s in range(c.n_cores)
             for j in range(h0, h1)]
    woT = np.ascontiguousarray(
        W_o.T.reshape(c.n_heads, c.dh, c.hidden)[order]
        .reshape(c.hidden, c.hidden)).astype(BF)   # [feat, out] full
    in_maps = []
    for g in range(c.n_cores):
        r0, r1 = g * c.FO, (g + 1) * c.FO
        wq = W_pack[r0:r1]
        wk = W_pack[H + r0:H + r1]
        wv = W_pack[2 * H + r0:2 * H + r1]
        wqkvT = np.ascontiguousarray(
            np.concatenate([wq, wk, wv], axis=0).T).astype(BF)  # [H, F]
        m = {"xt": XT, "wqkvt": wqkvT, "wot": woT}
        if mode == "masked":
            m["maskt"] = np.ascontiguousarray(mask.T * math.sqrt(c.dh))
        in_maps.append(m)
    return mode, in_maps


def assemble_output(cfg: Cfg, results):
    c = cfg
    full = np.empty((c.B, c.S, c.hidden), dtype=np.float32)
    for g in range(c.n_cores):
        o = results[g]["out"].reshape(c.B, c.TSH, c.hidden)
        for b in range(c.B):
            full[b, g * c.TSH:(g + 1) * c.TSH] = o[b]
    return full


def kernel(hidden_states, attention_mask, W_pack, W_o):
    cfg = Cfg()
    mode, in_maps = prepare_inputs(cfg, hidden_states, attention_mask,
                                   W_pack, W_o)
    nc = _get_program(cfg, mode)
    res = bass_utils.run_bass_kernel_spmd(nc, in_maps,
                                          list(range(cfg.n_cores)))
    return assemble_output(cfg, res.results)
